# revision 32
# baseline (speedup 1.0000x reference)
"""AttentionalPropagation (SuperGlue-style) fused Trainium2 kernel.

Full (unsharded) inputs -> full output. Internally: data-parallel over the
batch dim across 8 NeuronCores (B=8 -> 1 batch element per core); the
BatchNorm statistics (mean/var over batch AND sequence) are combined with a
tiny [128, 8] AllReduce.

Per core (b = core id), with D=256, N=2048, H=4 heads, hd=64:
  q = Wq x_b + bq ; k = Wk s_b + bk                (channels on partitions)
  vT = s_b^T Wv^T + bv (transposed projection -> m on partitions), plus a
       ones column per head that later yields softmax denominators for free
  per head: S^T = k_h^T q_h (PE) ; P^T = exp(S^T/8) (ACT, two m-tiles per
       op); attn_unnorm plus Z = vT_aug^T P^T (PE, accumulated over m);
       attn = attn_unnorm * bcast(1/Z) (DVE mult; 1/Z replicated across
       partitions by a broadcast DMA)
  message = Wm attn + bm
  h = W1x x_b + W1m message + b1 ; partial BN stats (sum, sum of squares)
  AllReduce stats ; scale/shift from mean/var ; hrelu = relu(h*s+t) (ACT)
  out_b = W2 hrelu + b2

All matmuls run in float32r (full PE rate); fp32 elsewhere. The attention
runs nb-outer / head-inner so the message/MLP matmuls for finished
n-blocks overlap the ACT-bound exp stream of later n-blocks.

Channel permutation: the torch module views D channels as (hd, H) so head h
owns channels {d*H+h}. All weight matrices are permuted on the host so
device channels are head-major (h*64+d); Wm's input side is permuted back.
"""

import os

import numpy as np

import concourse.bacc as bacc
import concourse.mybir as mybir
import concourse.tile as tile
from concourse.bass_utils import run_bass_kernel_spmd

B, D, N = 8, 256, 2048
H, HD = 4, 64
DD = 2 * D
EPS = 1e-3
N_CORES = 8
P = 128
NB = N // 512          # 4 n-blocks of 512
MT = N // P            # 16 m-tiles of 128
OT = D // P            # 2 channel tiles
OT2 = DD // P          # 4 hidden channel tiles
VW = 66                # per-head block in vT: 64 d cols + 1 ones col + 1 pad
VF = H * VW            # 264

f32 = mybir.dt.float32
f32r = mybir.dt.float32r
AF = mybir.ActivationFunctionType
ALU = mybir.AluOpType

_cache = {}


def _build(variant="full"):
    nc = bacc.Bacc("TRN2", num_devices=N_CORES)

    xb = nc.dram_tensor("xb", [P, OT * N], f32, kind="ExternalInput")
    sb = nc.dram_tensor("sb", [P, OT * N], f32, kind="ExternalInput")
    qw_d = nc.dram_tensor("qw", [P, OT * D], f32, kind="ExternalInput")
    kw_d = nc.dram_tensor("kw", [P, OT * D], f32, kind="ExternalInput")
    vw_d = nc.dram_tensor("vw", [P, OT * VF], f32, kind="ExternalInput")
    mw_d = nc.dram_tensor("mw", [P, OT * D], f32, kind="ExternalInput")
    w1x_d = nc.dram_tensor("w1x", [P, OT * DD], f32, kind="ExternalInput")
    w1m_d = nc.dram_tensor("w1m", [P, OT * DD], f32, kind="ExternalInput")
    w2_d = nc.dram_tensor("w2", [P, OT2 * D], f32, kind="ExternalInput")
    bva_d = nc.dram_tensor("bva", [1, VF], f32, kind="ExternalInput")
    vecs_d = nc.dram_tensor("vecs", [P, 20], f32, kind="ExternalInput")
    out_d = nc.dram_tensor("out", [P, OT * N], f32, kind="ExternalOutput")

    ones_d = nc.inline_tensor(np.ones((1, P), np.float32), name="ones_row")

    # vecs columns
    C_BQ, C_BK, C_BM, C_B1, C_GA, C_BE, C_B2 = 0, 2, 4, 6, 10, 14, 18

    with tile.TileContext(nc) as tc:
        with (
            tc.tile_pool(name="stage", bufs=2) as stage,       # f32 DMA landing
            tc.tile_pool(name="big", bufs=1) as big,           # 2MB-class f32r
            tc.tile_pool(name="wpool", bufs=1) as wpool,       # weights f32r
            tc.tile_pool(name="hpool", bufs=1) as hpool,       # h f32
            tc.tile_pool(name="ppool", bufs=4) as ppool,       # exp'd P^T tiles
            tc.tile_pool(name="small", bufs=2) as small,       # small work tiles
            tc.tile_pool(name="outp", bufs=2) as outp,         # output staging
            tc.tile_pool(name="ps2", bufs=1, space="PSUM") as ps2,   # 2-bank groups
            tc.tile_pool(name="ps_a", bufs=2, space="PSUM") as ps_a,  # attn accum
            tc.tile_pool(name="dram", bufs=1, space="DRAM") as dram,
        ):
            def load_round(dram_t, shape, name, pool=wpool, tag=None,
                           chunk=2048, eng=None):
                """sync-DMA f32 from DRAM then round into an f32r tile."""
                width = int(np.prod(shape[1:]))
                tf = pool.tile([shape[0], width], f32r, name=name,
                               tag=tag or name)
                flat_t = tf[:]
                if len(shape) == 3:
                    t = tf[:].rearrange("p (a b) -> p a b", a=shape[1])
                else:
                    t = tf[:]
                for lo in range(0, width, chunk):
                    hi = min(lo + chunk, width)
                    st = stage.tile([shape[0], min(chunk, width)], f32,
                                    tag="stage", name=f"st_{name}_{lo}")
                    nc.sync.dma_start(st[:, :hi - lo], dram_t[:, lo:hi])
                    if eng == "act":
                        nc.scalar.copy(flat_t[:, lo:hi], st[:, :hi - lo])
                    else:
                        nc.vector.tensor_copy(flat_t[:, lo:hi],
                                              st[:, :hi - lo])
                return t

            # ---- load + round inputs/weights ----
            qw = load_round(qw_d, [P, OT, D], "qw")
            kw = load_round(kw_d, [P, OT, D], "kw")
            vw = load_round(vw_d, [P, OT, VF], "vw")
            x_r = load_round(xb, [P, OT, N], "x_r", big, eng="act")
            s_r = load_round(sb, [P, OT, N], "s_r", big, tag="pAB",
                             eng="act")
            mw = load_round(mw_d, [P, OT, D], "mw")
            w1x = load_round(w1x_d, [P, OT, DD], "w1x")
            w1m = load_round(w1m_d, [P, OT, DD], "w1m")
            w2 = load_round(w2_d, [P, OT2, D], "w2")
            bva = load_round(bva_d, [1, VF], "bva")
            ones_r = load_round(ones_d, [1, P], "ones_r")

            vecs = small.tile([P, 20], f32, name="vecs", bufs=1)
            nc.sync.dma_start(vecs[:], vecs_d[:])

            # ---- q, k projections (channels on partitions, head-major) ----
            def proj(dst_name, w_t, rhs_t, bias_col, tag=None, dst=None):
                if dst is None:
                    dst = big.tile([P, OT, N], f32r, name=dst_name,
                                   tag=tag or dst_name)
                for ot in range(OT):
                    for nbp in range(NB // 2):
                        ps = ps2.tile([P, 2, 512], f32, tag="score", bufs=2,
                                      name=f"ps_{dst_name}_{ot}_{nbp}")
                        for j in range(2):
                            nb = 2 * nbp + j
                            for kt in range(OT):
                                nc.tensor.matmul(
                                    ps[:, j, :],
                                    w_t[:, kt, ot * P:(ot + 1) * P],
                                    rhs_t[:, kt, nb * 512:(nb + 1) * 512],
                                    start=(kt == 0), stop=(kt == OT - 1),
                                )
                        nc.vector.tensor_scalar_add(
                            dst[:, ot, nbp * 1024:(nbp + 1) * 1024],
                            ps[:].rearrange("p a b -> p (a b)"),
                            vecs[:, bias_col + ot: bias_col + ot + 1],
                        )
                return dst

            q_t = proj("q_t", qw, x_r, C_BQ, tag="pCD")
            k_t = proj("k_t", kw, s_r, C_BK)

            # ---- vT via transposed projection ----
            # vT[m, h*66+d] = sum_i source[i, m] WvT[i, h*66+d] + bva
            vt = wpool.tile([P, MT, VF], f32r, name="vt", tag="vt")
            for mt in range(MT):
                ps = ps2.tile([P, 512], f32, tag="work", bufs=2,
                              name=f"ps_vt_{mt}")
                for kt in range(OT):
                    nc.tensor.matmul(
                        ps[:, :VF], s_r[:, kt, mt * P:(mt + 1) * P],
                        vw[:, kt, :],
                        start=(kt == 0), stop=False,
                    )
                # bias row + ones columns via K=1 matmul of ones^T x bva
                nc.tensor.matmul(ps[:, :VF], ones_r[:], bva[:],
                                 start=False, stop=True)
                nc.vector.tensor_copy(vt[:, mt, :], ps[:, :VF])

            # ---- attention (nb-outer, head-inner) + message + MLP layer 1 ----
            attn = big.tile([P, OT, N], f32r, name="attn", tag="pAB")
            msg = big.tile([P, OT, N], f32r, name="msg", tag="pCD")
            h_t = hpool.tile([P, OT2, N], f32, name="h_t", tag="h_t")
            s1p = small.tile([P, OT2, NB], f32, name="s1p", bufs=1)
            s2p = small.tile([P, OT2, NB], f32, name="s2p", bufs=1)

            for nb in range(NB):
                nsl = slice(nb * 512, (nb + 1) * 512)
                for h in range(H):
                    pb, po = 64 * (h % 2), h // 2
                    pa = ps_a.tile([65, 512], f32, tag="ps_a",
                                   name=f"pa_{h}_{nb}")
                    for mtp in range(MT // 2):
                        pss = ps2.tile([P, 2, 512], f32, tag="score", bufs=2,
                                       name=f"pss_{h}_{nb}_{mtp}")
                        for j in range(2):
                            mt = 2 * mtp + j
                            nc.tensor.matmul(
                                pss[:, j, :],
                                k_t[pb:pb + 64, po, mt * P:(mt + 1) * P],
                                q_t[pb:pb + 64, po, nsl],
                                start=True, stop=True,
                            )
                        pt = ppool.tile([P, 2, 512], f32r, tag="pt",
                                        name=f"pt_{h}_{nb}_{mtp}")
                        nc.scalar.activation(
                            pt[:].rearrange("p a b -> p (a b)"),
                            pss[:].rearrange("p a b -> p (a b)"),
                            AF.Exp, scale=0.125,
                        )
                        for j in range(2):
                            mt = 2 * mtp + j
                            nc.tensor.matmul(
                                pa[:], vt[:, mt, h * VW:h * VW + 65],
                                pt[:, j, :],
                                start=(mtp == 0 and j == 0),
                                stop=(mtp == MT // 2 - 1 and j == 1),
                            )
                    rz = small.tile([1, 512], f32r, tag="rz",
                                    name=f"rz_{h}_{nb}", bufs=3)
                    with nc.allow_low_precision(reason="1/Z rounded to f32r"):
                        nc.vector.reciprocal(rz[:], pa[64:65, :])
                    zd = dram.tile([1, 512], f32r, tag="zd",
                                   name=f"zd_{h}_{nb}", bufs=2)
                    nc.sync.dma_start(zd[:], rz[:])
                    rzb = small.tile([64, 512], f32r, tag="rzb",
                                     name=f"rzb_{h}_{nb}", bufs=3)
                    nc.sync.dma_start(rzb[:], zd[:].to_broadcast((64, 512)))
                    nc.vector.tensor_tensor(
                        attn[pb:pb + 64, po, nsl], pa[0:64, :], rzb[:],
                        ALU.mult,
                    )

                # message for this n-block
                for ot in range(OT):
                    psm = ps2.tile([P, 512], f32, tag="work", bufs=2,
                                   name=f"ps_m_{ot}_{nb}")
                    for kt in range(OT):
                        nc.tensor.matmul(
                            psm[:], mw[:, kt, ot * P:(ot + 1) * P],
                            attn[:, kt, nsl],
                            start=(kt == 0), stop=(kt == OT - 1),
                        )
                    nc.vector.tensor_scalar_add(
                        msg[:, ot, nsl], psm[:],
                        vecs[:, C_BM + ot: C_BM + ot + 1],
                    )

                # MLP first layer + BN partial stats for this n-block
                for ot in range(OT2):
                    psh = ps2.tile([P, 512], f32, tag="work", bufs=2,
                                   name=f"ps_h_{ot}_{nb}")
                    for kt in range(OT):
                        nc.tensor.matmul(
                            psh[:], w1x[:, kt, ot * P:(ot + 1) * P],
                            x_r[:, kt, nsl],
                            start=(kt == 0), stop=False,
                        )
                    for kt in range(OT):
                        nc.tensor.matmul(
                            psh[:], w1m[:, kt, ot * P:(ot + 1) * P],
                            msg[:, kt, nsl],
                            start=False, stop=(kt == OT - 1),
                        )
                    hsl = h_t[:, ot, nsl]
                    nc.vector.tensor_scalar(
                        hsl, psh[:],
                        vecs[:, C_B1 + ot: C_B1 + ot + 1], None,
                        ALU.add, ALU.add,
                        accum_out=s1p[:, ot, nb:nb + 1],
                    )
                    sq = stage.tile([P, 512], f32, tag="sq",
                                    name="sq_scratch", bufs=2)
                    nc.scalar.activation(
                        sq[:], hsl, AF.Square,
                        accum_out=s2p[:, ot, nb:nb + 1],
                    )

            # ---- fold partials, AllReduce, scale/shift ----
            stats = small.tile([P, 8], f32, name="stats", bufs=1)
            for ot in range(OT2):
                nc.vector.reduce_sum(stats[:, ot:ot + 1], s1p[:, ot, :],
                                     axis=mybir.AxisListType.X)
                nc.vector.reduce_sum(stats[:, 4 + ot:5 + ot], s2p[:, ot, :],
                                     axis=mybir.AxisListType.X)

            if variant == "v3":
                gstats = stats  # local stats only (debug: skip collective)
            else:
                cin = dram.tile([P, 8], f32, name="cc_in")
                cout = dram.tile([P, 8], f32, addr_space="Shared",
                                 name="cc_out")
                nc.sync.dma_start(cin[:], stats[:])
                nc.gpsimd.collective_compute(
                    "AllReduce", ALU.add,
                    replica_groups=[list(range(N_CORES))],
                    ins=[cin[:].opt()], outs=[cout[:].opt()],
                )
                gstats = small.tile([P, 8], f32, name="gstats", bufs=1)
                nc.sync.dma_start(gstats[:], cout[:])

            inv_n = 1.0 / (B * N) if variant != "v3" else 1.0 / N
            mean = small.tile([P, 4], f32, name="mean", bufs=1)
            var = small.tile([P, 4], f32, name="var", bufs=1)
            scl = small.tile([P, 4], f32, name="scl", bufs=1)
            sft = small.tile([P, 4], f32, name="sft", bufs=1)
            nc.vector.tensor_scalar_mul(mean[:], gstats[:, 0:4], inv_n)
            nc.vector.tensor_scalar_mul(var[:], gstats[:, 4:8], inv_n)
            # var = E[h^2] - mean^2 + EPS
            nc.vector.tensor_tensor(scl[:], mean[:], mean[:], ALU.mult)
            nc.vector.tensor_tensor(var[:], var[:], scl[:], ALU.subtract)
            nc.vector.tensor_scalar_add(var[:], var[:], EPS)
            # rstd = exp(-0.5*ln(var)); scale = gamma*rstd; shift = beta-mean*scl
            nc.scalar.activation(scl[:], var[:], AF.Ln)
            nc.scalar.activation(scl[:], scl[:], AF.Exp, scale=-0.5)
            nc.vector.tensor_tensor(scl[:], scl[:], vecs[:, C_GA:C_GA + 4],
                                    ALU.mult)
            nc.vector.tensor_tensor(sft[:], mean[:], scl[:], ALU.mult)
            nc.vector.tensor_tensor(sft[:], vecs[:, C_BE:C_BE + 4], sft[:],
                                    ALU.subtract)

            # ---- BN apply + relu (f32r tiles reusing dead slots) ----
            hr_lo = big.tile([P, OT, N], f32r, name="hr_lo", tag="x_r")
            hr_hi = big.tile([P, OT, N], f32r, name="hr_hi", tag="k_t")
            hrelu = [hr_lo, hr_hi]
            for ot in range(OT2):
                dst = hrelu[ot // 2][:, ot % 2, :]
                if ot < 2:
                    nc.scalar.activation(
                        dst, h_t[:, ot, :], AF.Relu,
                        bias=sft[:, ot:ot + 1], scale=scl[:, ot:ot + 1],
                    )
                else:
                    nc.vector.tensor_scalar(
                        dst, h_t[:, ot, :], scl[:, ot:ot + 1],
                        sft[:, ot:ot + 1], ALU.mult, ALU.add,
                    )
                    nc.vector.tensor_scalar_max(dst, dst, 0.0)

            # ---- output projection ----
            for ot in range(OT):
                for nb in range(NB):
                    ps = ps2.tile([P, 512], f32, tag="work", bufs=2,
                                  name=f"ps_o_{ot}_{nb}")
                    for kt in range(OT2):
                        nc.tensor.matmul(
                            ps[:], w2[:, kt, ot * P:(ot + 1) * P],
                            hrelu[kt // 2][:, kt % 2,
                                           nb * 512:(nb + 1) * 512],
                            start=(kt == 0), stop=(kt == OT2 - 1),
                        )
                    ot_sb = outp.tile([P, 512], f32, tag="out_sb",
                                      name=f"osb_{ot}_{nb}")
                    nc.vector.tensor_scalar_add(
                        ot_sb[:], ps[:],
                        vecs[:, C_B2 + ot: C_B2 + ot + 1],
                    )
                    nc.sync.dma_start(
                        out_d[:, ot * N + nb * 512: ot * N + (nb + 1) * 512],
                        ot_sb[:],
                    )

    nc.compile()
    return nc


def _prep(inputs):
    """Host-side numpy prep: permutations, transposes, tiling."""
    perm = np.array([d * H + h for h in range(H) for d in range(HD)])

    def tile_kxm(wt):  # [K, M] -> [P, (K//P)*M] with kt-major columns
        k, m = wt.shape
        return np.ascontiguousarray(
            wt.reshape(k // P, P, m).transpose(1, 0, 2).reshape(P, (k // P) * m)
        )

    Wq, Wk, Wv, Wm = inputs["Wq"], inputs["Wk"], inputs["Wv"], inputs["Wm"]
    bq, bk, bv, bm = inputs["bq"], inputs["bk"], inputs["bv"], inputs["bm"]
    W1, b1, W2, b2 = inputs["W1"], inputs["b1"], inputs["W2"], inputs["b2"]
    gamma1, beta1 = inputs["gamma1"], inputs["beta1"]

    qw = tile_kxm(np.asarray(Wq)[perm, :].T)
    kw = tile_kxm(np.asarray(Wk)[perm, :].T)
    mw = tile_kxm(np.asarray(Wm)[:, perm].T)

    wv_aug = np.zeros((D, VF), np.float32)
    bva = np.zeros((1, VF), np.float32)
    WvTp = np.asarray(Wv)[perm, :].T
    bvp = np.asarray(bv)[perm]
    for h in range(H):
        wv_aug[:, h * VW:h * VW + HD] = WvTp[:, h * HD:(h + 1) * HD]
        bva[0, h * VW:h * VW + HD] = bvp[h * HD:(h + 1) * HD]
        bva[0, h * VW + HD] = 1.0
    vw = tile_kxm(wv_aug)

    w1x = tile_kxm(np.asarray(W1)[:, :D].T)
    w1m = tile_kxm(np.asarray(W1)[:, D:].T)
    w2 = tile_kxm(np.asarray(W2).T)

    def cols(v):  # [C] -> [P, C//P] channel-tiled per-partition columns
        return np.asarray(v).reshape(-1, P).T

    vecs = np.concatenate(
        [cols(np.asarray(bq)[perm]), cols(np.asarray(bk)[perm]), cols(bm),
         cols(b1), cols(gamma1), cols(beta1), cols(b2)], axis=1,
    ).astype(np.float32)

    def tile_x(t):  # [B, D, N] -> [B, P, OT*N]
        return np.ascontiguousarray(
            t.reshape(B, OT, P, N).transpose(0, 2, 1, 3).reshape(B, P, OT * N)
        )

    xb = tile_x(np.asarray(inputs["x"], np.float32))
    sb = tile_x(np.asarray(inputs["source"], np.float32))

    shared = {
        "qw": qw, "kw": kw, "vw": vw, "mw": mw,
        "w1x": w1x, "w1m": w1m, "w2": w2,
        "bva": bva, "vecs": np.ascontiguousarray(vecs),
    }
    shared = {k: np.ascontiguousarray(v.astype(np.float32))
              for k, v in shared.items()}
    return [
        {**shared, "xb": np.ascontiguousarray(xb[c]),
         "sb": np.ascontiguousarray(sb[c])}
        for c in range(N_CORES)
    ]


def _make_runner(nc):
    """Build the sharded PJRT executable ONCE; reuse across kernel() calls."""
    import jax
    import concourse.mybir as _mybir
    from concourse import bass2jax
    from jax.experimental.shard_map import shard_map
    from jax.sharding import Mesh, PartitionSpec

    bass2jax.install_neuronx_cc_hook()

    partition_name = (nc.partition_id_tensor.name
                      if nc.partition_id_tensor else None)
    in_names, out_names, out_avals, zero_outs = [], [], [], []
    for alloc in nc.m.functions[0].allocations:
        if not isinstance(alloc, _mybir.MemoryLocationSet):
            continue
        name = alloc.memorylocations[0].name
        if alloc.kind == "ExternalInput":
            if name != partition_name:
                in_names.append(name)
        elif alloc.kind == "ExternalOutput":
            out_names.append(name)
            shape = tuple(alloc.tensor_shape)
            dtype = _mybir.dt.np(alloc.dtype)
            out_avals.append(jax.core.ShapedArray(shape, dtype))
            zero_outs.append(np.zeros(shape, dtype))
    n_params = len(in_names)
    n_outs = len(out_avals)
    all_in_names = list(in_names) + list(out_names)
    if partition_name is not None:
        all_in_names.append(partition_name)
    donate = tuple(range(n_params, n_params + n_outs))

    def _body(*args):
        operands = list(args)
        if partition_name is not None:
            operands.append(bass2jax.partition_id_tensor())
        outs = bass2jax._bass_exec_p.bind(
            *operands,
            out_avals=tuple(out_avals),
            in_names=tuple(all_in_names),
            out_names=tuple(out_names),
            lowering_input_output_aliases=(),
            sim_require_finite=True,
            sim_require_nnan=True,
            nc=nc,
        )
        return tuple(outs)

    devices = jax.devices()[:N_CORES]
    mesh = Mesh(np.asarray(devices), ("core",))
    in_specs = (PartitionSpec("core"),) * (n_params + n_outs)
    out_specs = (PartitionSpec("core"),) * n_outs
    sharded = jax.jit(
        shard_map(_body, mesh=mesh, in_specs=in_specs, out_specs=out_specs,
                  check_rep=False),
        keep_unused=True,
    )
    from jax.sharding import NamedSharding
    core_sh = NamedSharding(mesh, PartitionSpec("core"))
    import hashlib
    dev_cache = {}
    # the kernel writes every output element, so the output-aliased operands
    # just need to exist; upload the zeros once and reuse them (no donation)
    dev_zeros = [
        jax.device_put(np.zeros((N_CORES * z.shape[0], *z.shape[1:]),
                                z.dtype), core_sh)
        for z in zero_outs
    ]

    def run(in_maps):
        ins = []
        for name in in_names:
            per_core = [np.asarray(in_maps[c][name]) for c in range(N_CORES)]
            same = all(p is per_core[0] for p in per_core)
            if same and per_core[0].nbytes <= 16 << 20:
                # replicated small tensor (weights): cache device-resident
                # copy keyed by content hash so repeat calls skip the upload
                key = (name, hashlib.blake2b(per_core[0].tobytes(),
                                             digest_size=16).digest())
                arr = dev_cache.get(key)
                if arr is None:
                    if len(dev_cache) > 64:
                        dev_cache.clear()
                    cat = np.concatenate(per_core, axis=0)
                    arr = jax.device_put(cat, core_sh)
                    dev_cache[key] = arr
                ins.append(arr)
            else:
                ins.append(np.concatenate(per_core, axis=0))
        out_arrs = sharded(*ins, *dev_zeros)
        return [
            {name: np.asarray(out_arrs[i]).reshape(
                N_CORES, *out_avals[i].shape)[c]
             for i, name in enumerate(out_names)}
            for c in range(N_CORES)
        ]

    return run


def _run(inputs, **kwargs):
    variant = os.environ.get("KERNEL_VARIANT", "full")
    key = ("nc", variant)
    if key not in _cache:
        _cache[key] = _build(variant)
    nc = _cache[key]
    in_maps = _prep(inputs)
    from concourse._compat import axon_active
    if kwargs or not axon_active():
        return run_bass_kernel_spmd(nc, in_maps,
                                    core_ids=list(range(N_CORES)), **kwargs)
    rkey = ("runner", variant)
    if rkey not in _cache:
        _cache[rkey] = _make_runner(nc)
    results = _cache[rkey](in_maps)

    class _R:
        pass

    res = _R()
    res.results = results
    return res


def _unpack(res):
    out = np.empty((B, D, N), np.float32)
    for c in range(N_CORES):
        o = res.results[c]["out"]  # [P, OT*N]
        out[c] = o.reshape(P, OT, N).transpose(1, 0, 2).reshape(D, N)
    return out


def kernel(**inputs):
    return _unpack(_run(inputs))


def run_traced(**inputs):
    """Dev helper: run with NTFF tracing; returns BassKernelResults."""
    return _run(inputs, trace=True)


# revision 37
# speedup vs baseline: 1.0346x; 1.0346x over previous
"""AttentionalPropagation (SuperGlue-style) fused Trainium2 kernel.

Full (unsharded) inputs -> full output. Internally: data-parallel over the
batch dim across 8 NeuronCores (B=8 -> 1 batch element per core); the
BatchNorm statistics (mean/var over batch AND sequence) are combined with a
tiny [128, 8] AllReduce.

Per core (b = core id), with D=256, N=2048, H=4 heads, hd=64:
  q = Wq x_b + bq ; k = Wk s_b + bk                (channels on partitions)
  vT = s_b^T Wv^T + bv (transposed projection -> m on partitions), plus a
       ones column per head that later yields softmax denominators for free
  per head: S^T = k_h^T q_h (PE) ; P^T = exp(S^T/8) (ACT, two m-tiles per
       op); attn_unnorm plus Z = vT_aug^T P^T (PE, accumulated over m);
       attn = attn_unnorm * bcast(1/Z) (DVE mult; 1/Z replicated across
       partitions by a broadcast DMA)
  message = Wm attn + bm
  h = W1x x_b + W1m message + b1 ; partial BN stats (sum, sum of squares)
  AllReduce stats ; scale/shift from mean/var ; hrelu = relu(h*s+t) (ACT)
  out_b = W2 hrelu + b2

All matmuls run in float32r (full PE rate); fp32 elsewhere. The attention
runs nb-outer / head-inner so the message/MLP matmuls for finished
n-blocks overlap the ACT-bound exp stream of later n-blocks.

Channel permutation: the torch module views D channels as (hd, H) so head h
owns channels {d*H+h}. All weight matrices are permuted on the host so
device channels are head-major (h*64+d); Wm's input side is permuted back.
"""

import os

import numpy as np

import concourse.bacc as bacc
import concourse.mybir as mybir
import concourse.tile as tile
from concourse.bass_utils import run_bass_kernel_spmd

B, D, N = 8, 256, 2048
H, HD = 4, 64
DD = 2 * D
EPS = 1e-3
N_CORES = 8
P = 128
NB = N // 512          # 4 n-blocks of 512
MT = N // P            # 16 m-tiles of 128
OT = D // P            # 2 channel tiles
OT2 = DD // P          # 4 hidden channel tiles
VW = 66                # per-head block in vT: 64 d cols + 1 ones col + 1 pad
VF = H * VW            # 264

f32 = mybir.dt.float32
f32r = mybir.dt.float32r
AF = mybir.ActivationFunctionType
ALU = mybir.AluOpType

_cache = {}


def _build(variant="full"):
    nc = bacc.Bacc("TRN2", num_devices=N_CORES)

    xb = nc.dram_tensor("xb", [P, OT * N], f32, kind="ExternalInput")
    sb = nc.dram_tensor("sb", [P, OT * N], f32, kind="ExternalInput")
    qw_d = nc.dram_tensor("qw", [P, OT * D], f32, kind="ExternalInput")
    kw_d = nc.dram_tensor("kw", [P, OT * D], f32, kind="ExternalInput")
    vw_d = nc.dram_tensor("vw", [P, OT * VF], f32, kind="ExternalInput")
    mw_d = nc.dram_tensor("mw", [P, OT * D], f32, kind="ExternalInput")
    w1x_d = nc.dram_tensor("w1x", [P, OT * DD], f32, kind="ExternalInput")
    w1m_d = nc.dram_tensor("w1m", [P, OT * DD], f32, kind="ExternalInput")
    w2_d = nc.dram_tensor("w2", [P, OT2 * D], f32, kind="ExternalInput")
    bva_d = nc.dram_tensor("bva", [1, VF], f32, kind="ExternalInput")
    vecs_d = nc.dram_tensor("vecs", [P, 20], f32, kind="ExternalInput")
    out_d = nc.dram_tensor("out", [P, OT * N], f32, kind="ExternalOutput")

    ones_d = nc.inline_tensor(np.ones((1, P), np.float32), name="ones_row")

    # vecs columns
    C_BQ, C_BK, C_BM, C_B1, C_GA, C_BE, C_B2 = 0, 2, 4, 6, 10, 14, 18

    with tile.TileContext(nc) as tc:
        with (
            tc.tile_pool(name="stage", bufs=3) as stage,       # f32 DMA landing
            tc.tile_pool(name="big", bufs=1) as big,           # 2MB-class f32r
            tc.tile_pool(name="wpool", bufs=1) as wpool,       # weights f32r
            tc.tile_pool(name="hpool", bufs=1) as hpool,       # h f32
            tc.tile_pool(name="ppool", bufs=4) as ppool,       # exp'd P^T tiles
            tc.tile_pool(name="small", bufs=2) as small,       # small work tiles
            tc.tile_pool(name="outp", bufs=3) as outp,         # output staging
            tc.tile_pool(name="ps2", bufs=1, space="PSUM") as ps2,   # 2-bank groups
            tc.tile_pool(name="ps_a", bufs=2, space="PSUM") as ps_a,  # attn accum
            tc.tile_pool(name="dram", bufs=1, space="DRAM") as dram,
        ):
            def load_round(dram_t, shape, name, pool=wpool, tag=None,
                           chunk=2048, eng=None):
                """sync-DMA f32 from DRAM then round into an f32r tile."""
                width = int(np.prod(shape[1:]))
                tf = pool.tile([shape[0], width], f32r, name=name,
                               tag=tag or name)
                flat_t = tf[:]
                if len(shape) == 3:
                    t = tf[:].rearrange("p (a b) -> p a b", a=shape[1])
                else:
                    t = tf[:]
                for lo in range(0, width, chunk):
                    hi = min(lo + chunk, width)
                    st = stage.tile([shape[0], min(chunk, width)], f32,
                                    tag="stage", name=f"st_{name}_{lo}")
                    nc.sync.dma_start(st[:, :hi - lo], dram_t[:, lo:hi])
                    if eng == "act":
                        nc.scalar.copy(flat_t[:, lo:hi], st[:, :hi - lo])
                    else:
                        nc.vector.tensor_copy(flat_t[:, lo:hi],
                                              st[:, :hi - lo])
                return t

            # ---- load + round inputs/weights ----
            qw = load_round(qw_d, [P, OT, D], "qw")
            kw = load_round(kw_d, [P, OT, D], "kw")
            vw = load_round(vw_d, [P, OT, VF], "vw")
            x_r = load_round(xb, [P, OT, N], "x_r", big, eng="act",
                             chunk=1024)
            s_r = load_round(sb, [P, OT, N], "s_r", big, tag="pAB",
                             eng="act", chunk=1024)
            mw = load_round(mw_d, [P, OT, D], "mw")
            w1x = load_round(w1x_d, [P, OT, DD], "w1x")
            w1m = load_round(w1m_d, [P, OT, DD], "w1m")
            w2 = load_round(w2_d, [P, OT2, D], "w2")
            bva = load_round(bva_d, [1, VF], "bva")
            ones_r = load_round(ones_d, [1, P], "ones_r")

            vecs = small.tile([P, 20], f32, name="vecs", bufs=1)
            nc.sync.dma_start(vecs[:], vecs_d[:])
            # warm the ACT exp table during startup so the ~2.7us table load
            # doesn't stall the first attention exp
            warm = small.tile([1, 16], f32, name="warm", bufs=1)
            nc.scalar.activation(warm[:], vecs[0:1, 0:16], AF.Exp)

            # ---- q, k projections (channels on partitions, head-major) ----
            def proj(dst_name, w_t, rhs_t, bias_col, tag=None, dst=None):
                if dst is None:
                    dst = big.tile([P, OT, N], f32r, name=dst_name,
                                   tag=tag or dst_name)
                for ot in range(OT):
                    for nbp in range(NB // 2):
                        ps = ps2.tile([P, 2, 512], f32, tag="score", bufs=2,
                                      name=f"ps_{dst_name}_{ot}_{nbp}")
                        for j in range(2):
                            nb = 2 * nbp + j
                            for kt in range(OT):
                                nc.tensor.matmul(
                                    ps[:, j, :],
                                    w_t[:, kt, ot * P:(ot + 1) * P],
                                    rhs_t[:, kt, nb * 512:(nb + 1) * 512],
                                    start=(kt == 0), stop=(kt == OT - 1),
                                )
                        nc.vector.tensor_scalar_add(
                            dst[:, ot, nbp * 1024:(nbp + 1) * 1024],
                            ps[:].rearrange("p a b -> p (a b)"),
                            vecs[:, bias_col + ot: bias_col + ot + 1],
                        )
                return dst

            q_t = proj("q_t", qw, x_r, C_BQ, tag="pCD")
            k_t = proj("k_t", kw, s_r, C_BK)

            # ---- vT via transposed projection ----
            # vT[m, h*66+d] = sum_i source[i, m] WvT[i, h*66+d] + bva
            vt = wpool.tile([P, MT, VF], f32r, name="vt", tag="vt")
            for mt in range(MT):
                ps = ps2.tile([P, 512], f32, tag="work", bufs=2,
                              name=f"ps_vt_{mt}")
                for kt in range(OT):
                    nc.tensor.matmul(
                        ps[:, :VF], s_r[:, kt, mt * P:(mt + 1) * P],
                        vw[:, kt, :],
                        start=(kt == 0), stop=False,
                    )
                # bias row + ones columns via K=1 matmul of ones^T x bva
                nc.tensor.matmul(ps[:, :VF], ones_r[:], bva[:],
                                 start=False, stop=True)
                nc.vector.tensor_copy(vt[:, mt, :], ps[:, :VF])

            # ---- attention (nb-outer, head-inner) + message + MLP layer 1 ----
            attn = big.tile([P, OT, N], f32r, name="attn", tag="pAB")
            msg = big.tile([P, OT, N], f32r, name="msg", tag="pCD")
            h_t = hpool.tile([P, OT2, N], f32, name="h_t", tag="h_t")
            s1p = small.tile([P, OT2, NB], f32, name="s1p", bufs=1)
            s2p = small.tile([P, OT2, NB], f32, name="s2p", bufs=1)

            for nb in range(NB):
                nsl = slice(nb * 512, (nb + 1) * 512)
                for h in range(H):
                    pb, po = 64 * (h % 2), h // 2
                    pa = ps_a.tile([65, 512], f32, tag="ps_a",
                                   name=f"pa_{h}_{nb}")
                    for mtp in range(MT // 2):
                        pss = ps2.tile([P, 2, 512], f32, tag="score", bufs=2,
                                       name=f"pss_{h}_{nb}_{mtp}")
                        for j in range(2):
                            mt = 2 * mtp + j
                            nc.tensor.matmul(
                                pss[:, j, :],
                                k_t[pb:pb + 64, po, mt * P:(mt + 1) * P],
                                q_t[pb:pb + 64, po, nsl],
                                start=True, stop=True,
                            )
                        pt = ppool.tile([P, 2, 512], f32r, tag="pt",
                                        name=f"pt_{h}_{nb}_{mtp}")
                        nc.scalar.activation(
                            pt[:].rearrange("p a b -> p (a b)"),
                            pss[:].rearrange("p a b -> p (a b)"),
                            AF.Exp, scale=0.125,
                        )
                        for j in range(2):
                            mt = 2 * mtp + j
                            nc.tensor.matmul(
                                pa[:], vt[:, mt, h * VW:h * VW + 65],
                                pt[:, j, :],
                                start=(mtp == 0 and j == 0),
                                stop=(mtp == MT // 2 - 1 and j == 1),
                            )
                    rz = small.tile([1, 512], f32r, tag="rz",
                                    name=f"rz_{h}_{nb}", bufs=3)
                    with nc.allow_low_precision(reason="1/Z rounded to f32r"):
                        nc.vector.reciprocal(rz[:], pa[64:65, :])
                    zd = dram.tile([1, 512], f32r, tag="zd",
                                   name=f"zd_{h}_{nb}", bufs=2)
                    nc.sync.dma_start(zd[:], rz[:])
                    rzb = small.tile([64, 512], f32r, tag="rzb",
                                     name=f"rzb_{h}_{nb}", bufs=3)
                    nc.sync.dma_start(rzb[:], zd[:].to_broadcast((64, 512)))
                    nc.vector.tensor_tensor(
                        attn[pb:pb + 64, po, nsl], pa[0:64, :], rzb[:],
                        ALU.mult,
                    )

                # message for this n-block
                for ot in range(OT):
                    psm = ps2.tile([P, 512], f32, tag="work", bufs=2,
                                   name=f"ps_m_{ot}_{nb}")
                    for kt in range(OT):
                        nc.tensor.matmul(
                            psm[:], mw[:, kt, ot * P:(ot + 1) * P],
                            attn[:, kt, nsl],
                            start=(kt == 0), stop=(kt == OT - 1),
                        )
                    nc.vector.tensor_scalar_add(
                        msg[:, ot, nsl], psm[:],
                        vecs[:, C_BM + ot: C_BM + ot + 1],
                    )

                # MLP first layer + BN partial stats for this n-block
                for ot in range(OT2):
                    psh = ps2.tile([P, 512], f32, tag="work", bufs=2,
                                   name=f"ps_h_{ot}_{nb}")
                    for kt in range(OT):
                        nc.tensor.matmul(
                            psh[:], w1x[:, kt, ot * P:(ot + 1) * P],
                            x_r[:, kt, nsl],
                            start=(kt == 0), stop=False,
                        )
                    for kt in range(OT):
                        nc.tensor.matmul(
                            psh[:], w1m[:, kt, ot * P:(ot + 1) * P],
                            msg[:, kt, nsl],
                            start=False, stop=(kt == OT - 1),
                        )
                    hsl = h_t[:, ot, nsl]
                    nc.vector.tensor_scalar(
                        hsl, psh[:],
                        vecs[:, C_B1 + ot: C_B1 + ot + 1], None,
                        ALU.add, ALU.add,
                        accum_out=s1p[:, ot, nb:nb + 1],
                    )
                    sq = stage.tile([P, 512], f32, tag="sq",
                                    name="sq_scratch", bufs=2)
                    nc.scalar.activation(
                        sq[:], hsl, AF.Square,
                        accum_out=s2p[:, ot, nb:nb + 1],
                    )

            # ---- fold partials, AllReduce, scale/shift ----
            stats = small.tile([P, 8], f32, name="stats", bufs=1)
            for ot in range(OT2):
                nc.vector.reduce_sum(stats[:, ot:ot + 1], s1p[:, ot, :],
                                     axis=mybir.AxisListType.X)
                nc.vector.reduce_sum(stats[:, 4 + ot:5 + ot], s2p[:, ot, :],
                                     axis=mybir.AxisListType.X)

            if variant == "v3":
                gstats = stats  # local stats only (debug: skip collective)
            else:
                cin = dram.tile([P, 8], f32, name="cc_in")
                cout = dram.tile([P, 8], f32, addr_space="Shared",
                                 name="cc_out")
                nc.sync.dma_start(cin[:], stats[:])
                nc.gpsimd.collective_compute(
                    "AllReduce", ALU.add,
                    replica_groups=[list(range(N_CORES))],
                    ins=[cin[:].opt()], outs=[cout[:].opt()],
                )
                gstats = small.tile([P, 8], f32, name="gstats", bufs=1)
                nc.sync.dma_start(gstats[:], cout[:])

            inv_n = 1.0 / (B * N) if variant != "v3" else 1.0 / N
            mean = small.tile([P, 4], f32, name="mean", bufs=1)
            var = small.tile([P, 4], f32, name="var", bufs=1)
            scl = small.tile([P, 4], f32, name="scl", bufs=1)
            sft = small.tile([P, 4], f32, name="sft", bufs=1)
            nc.vector.tensor_scalar_mul(mean[:], gstats[:, 0:4], inv_n)
            nc.vector.tensor_scalar_mul(var[:], gstats[:, 4:8], inv_n)
            # var = E[h^2] - mean^2 + EPS
            nc.vector.tensor_tensor(scl[:], mean[:], mean[:], ALU.mult)
            nc.vector.tensor_tensor(var[:], var[:], scl[:], ALU.subtract)
            nc.vector.tensor_scalar_add(var[:], var[:], EPS)
            # rstd = exp(-0.5*ln(var)); scale = gamma*rstd; shift = beta-mean*scl
            nc.scalar.activation(scl[:], var[:], AF.Ln)
            nc.scalar.activation(scl[:], scl[:], AF.Exp, scale=-0.5)
            nc.vector.tensor_tensor(scl[:], scl[:], vecs[:, C_GA:C_GA + 4],
                                    ALU.mult)
            nc.vector.tensor_tensor(sft[:], mean[:], scl[:], ALU.mult)
            nc.vector.tensor_tensor(sft[:], vecs[:, C_BE:C_BE + 4], sft[:],
                                    ALU.subtract)

            # ---- BN apply + relu (f32r tiles reusing dead slots) ----
            hr_lo = big.tile([P, OT, N], f32r, name="hr_lo", tag="x_r")
            hr_hi = big.tile([P, OT, N], f32r, name="hr_hi", tag="k_t")
            hrelu = [hr_lo, hr_hi]
            for ot in range(OT2):
                dst = hrelu[ot // 2][:, ot % 2, :]
                if ot < 2:
                    nc.scalar.activation(
                        dst, h_t[:, ot, :], AF.Relu,
                        bias=sft[:, ot:ot + 1], scale=scl[:, ot:ot + 1],
                    )
                else:
                    nc.vector.tensor_scalar(
                        dst, h_t[:, ot, :], scl[:, ot:ot + 1],
                        sft[:, ot:ot + 1], ALU.mult, ALU.add,
                    )
                    nc.vector.tensor_scalar_max(dst, dst, 0.0)

            # ---- output projection ----
            for ot in range(OT):
                for nb in range(NB):
                    ps = ps2.tile([P, 512], f32, tag="work", bufs=2,
                                  name=f"ps_o_{ot}_{nb}")
                    for kt in range(OT2):
                        nc.tensor.matmul(
                            ps[:], w2[:, kt, ot * P:(ot + 1) * P],
                            hrelu[kt // 2][:, kt % 2,
                                           nb * 512:(nb + 1) * 512],
                            start=(kt == 0), stop=(kt == OT2 - 1),
                        )
                    ot_sb = outp.tile([P, 512], f32, tag="out_sb",
                                      name=f"osb_{ot}_{nb}")
                    nc.vector.tensor_scalar_add(
                        ot_sb[:], ps[:],
                        vecs[:, C_B2 + ot: C_B2 + ot + 1],
                    )
                    nc.sync.dma_start(
                        out_d[:, ot * N + nb * 512: ot * N + (nb + 1) * 512],
                        ot_sb[:],
                    )

    nc.compile()
    return nc


def _prep(inputs):
    """Host-side numpy prep: permutations, transposes, tiling."""
    perm = np.array([d * H + h for h in range(H) for d in range(HD)])

    def tile_kxm(wt):  # [K, M] -> [P, (K//P)*M] with kt-major columns
        k, m = wt.shape
        return np.ascontiguousarray(
            wt.reshape(k // P, P, m).transpose(1, 0, 2).reshape(P, (k // P) * m)
        )

    Wq, Wk, Wv, Wm = inputs["Wq"], inputs["Wk"], inputs["Wv"], inputs["Wm"]
    bq, bk, bv, bm = inputs["bq"], inputs["bk"], inputs["bv"], inputs["bm"]
    W1, b1, W2, b2 = inputs["W1"], inputs["b1"], inputs["W2"], inputs["b2"]
    gamma1, beta1 = inputs["gamma1"], inputs["beta1"]

    qw = tile_kxm(np.asarray(Wq)[perm, :].T)
    kw = tile_kxm(np.asarray(Wk)[perm, :].T)
    mw = tile_kxm(np.asarray(Wm)[:, perm].T)

    wv_aug = np.zeros((D, VF), np.float32)
    bva = np.zeros((1, VF), np.float32)
    WvTp = np.asarray(Wv)[perm, :].T
    bvp = np.asarray(bv)[perm]
    for h in range(H):
        wv_aug[:, h * VW:h * VW + HD] = WvTp[:, h * HD:(h + 1) * HD]
        bva[0, h * VW:h * VW + HD] = bvp[h * HD:(h + 1) * HD]
        bva[0, h * VW + HD] = 1.0
    vw = tile_kxm(wv_aug)

    w1x = tile_kxm(np.asarray(W1)[:, :D].T)
    w1m = tile_kxm(np.asarray(W1)[:, D:].T)
    w2 = tile_kxm(np.asarray(W2).T)

    def cols(v):  # [C] -> [P, C//P] channel-tiled per-partition columns
        return np.asarray(v).reshape(-1, P).T

    vecs = np.concatenate(
        [cols(np.asarray(bq)[perm]), cols(np.asarray(bk)[perm]), cols(bm),
         cols(b1), cols(gamma1), cols(beta1), cols(b2)], axis=1,
    ).astype(np.float32)

    def tile_x(t):  # [B, D, N] -> [B, P, OT*N]
        return np.ascontiguousarray(
            t.reshape(B, OT, P, N).transpose(0, 2, 1, 3).reshape(B, P, OT * N)
        )

    xb = tile_x(np.asarray(inputs["x"], np.float32))
    sb = tile_x(np.asarray(inputs["source"], np.float32))

    shared = {
        "qw": qw, "kw": kw, "vw": vw, "mw": mw,
        "w1x": w1x, "w1m": w1m, "w2": w2,
        "bva": bva, "vecs": np.ascontiguousarray(vecs),
    }
    shared = {k: np.ascontiguousarray(v.astype(np.float32))
              for k, v in shared.items()}
    return [
        {**shared, "xb": np.ascontiguousarray(xb[c]),
         "sb": np.ascontiguousarray(sb[c])}
        for c in range(N_CORES)
    ]


def _make_runner(nc):
    """Build the sharded PJRT executable ONCE; reuse across kernel() calls."""
    import jax
    import concourse.mybir as _mybir
    from concourse import bass2jax
    from jax.experimental.shard_map import shard_map
    from jax.sharding import Mesh, PartitionSpec

    bass2jax.install_neuronx_cc_hook()

    partition_name = (nc.partition_id_tensor.name
                      if nc.partition_id_tensor else None)
    in_names, out_names, out_avals, zero_outs = [], [], [], []
    for alloc in nc.m.functions[0].allocations:
        if not isinstance(alloc, _mybir.MemoryLocationSet):
            continue
        name = alloc.memorylocations[0].name
        if alloc.kind == "ExternalInput":
            if name != partition_name:
                in_names.append(name)
        elif alloc.kind == "ExternalOutput":
            out_names.append(name)
            shape = tuple(alloc.tensor_shape)
            dtype = _mybir.dt.np(alloc.dtype)
            out_avals.append(jax.core.ShapedArray(shape, dtype))
            zero_outs.append(np.zeros(shape, dtype))
    n_params = len(in_names)
    n_outs = len(out_avals)
    all_in_names = list(in_names) + list(out_names)
    if partition_name is not None:
        all_in_names.append(partition_name)
    donate = tuple(range(n_params, n_params + n_outs))

    def _body(*args):
        operands = list(args)
        if partition_name is not None:
            operands.append(bass2jax.partition_id_tensor())
        outs = bass2jax._bass_exec_p.bind(
            *operands,
            out_avals=tuple(out_avals),
            in_names=tuple(all_in_names),
            out_names=tuple(out_names),
            lowering_input_output_aliases=(),
            sim_require_finite=True,
            sim_require_nnan=True,
            nc=nc,
        )
        return tuple(outs)

    devices = jax.devices()[:N_CORES]
    mesh = Mesh(np.asarray(devices), ("core",))
    in_specs = (PartitionSpec("core"),) * (n_params + n_outs)
    out_specs = (PartitionSpec("core"),) * n_outs
    sharded = jax.jit(
        shard_map(_body, mesh=mesh, in_specs=in_specs, out_specs=out_specs,
                  check_rep=False),
        keep_unused=True,
    )
    from jax.sharding import NamedSharding
    core_sh = NamedSharding(mesh, PartitionSpec("core"))
    import hashlib
    dev_cache = {}
    # the kernel writes every output element, so the output-aliased operands
    # just need to exist; upload the zeros once and reuse them (no donation)
    dev_zeros = [
        jax.device_put(np.zeros((N_CORES * z.shape[0], *z.shape[1:]),
                                z.dtype), core_sh)
        for z in zero_outs
    ]

    def run(in_maps):
        ins = []
        for name in in_names:
            per_core = [np.asarray(in_maps[c][name]) for c in range(N_CORES)]
            same = all(p is per_core[0] for p in per_core)
            if same and per_core[0].nbytes <= 16 << 20:
                # replicated small tensor (weights): cache device-resident
                # copy keyed by content hash so repeat calls skip the upload
                key = (name, hashlib.blake2b(per_core[0].tobytes(),
                                             digest_size=16).digest())
                arr = dev_cache.get(key)
                if arr is None:
                    if len(dev_cache) > 64:
                        dev_cache.clear()
                    cat = np.concatenate(per_core, axis=0)
                    arr = jax.device_put(cat, core_sh)
                    dev_cache[key] = arr
                ins.append(arr)
            else:
                ins.append(np.concatenate(per_core, axis=0))
        out_arrs = sharded(*ins, *dev_zeros)
        return [
            {name: np.asarray(out_arrs[i]).reshape(
                N_CORES, *out_avals[i].shape)[c]
             for i, name in enumerate(out_names)}
            for c in range(N_CORES)
        ]

    return run


def _run(inputs, **kwargs):
    variant = os.environ.get("KERNEL_VARIANT", "full")
    key = ("nc", variant)
    if key not in _cache:
        _cache[key] = _build(variant)
    nc = _cache[key]
    in_maps = _prep(inputs)
    from concourse._compat import axon_active
    if kwargs or not axon_active():
        return run_bass_kernel_spmd(nc, in_maps,
                                    core_ids=list(range(N_CORES)), **kwargs)
    rkey = ("runner", variant)
    if rkey not in _cache:
        _cache[rkey] = _make_runner(nc)
    results = _cache[rkey](in_maps)

    class _R:
        pass

    res = _R()
    res.results = results
    return res


def _unpack(res):
    out = np.empty((B, D, N), np.float32)
    for c in range(N_CORES):
        o = res.results[c]["out"]  # [P, OT*N]
        out[c] = o.reshape(P, OT, N).transpose(1, 0, 2).reshape(D, N)
    return out


def kernel(**inputs):
    return _unpack(_run(inputs))


def run_traced(**inputs):
    """Dev helper: run with NTFF tracing; returns BassKernelResults."""
    return _run(inputs, trace=True)
